# revision 6
# baseline (speedup 1.0000x reference)
"""Trainium2 Bass kernel for nn_ChannelGroupConvUneven.

Computes, for full inputs
    x      (8, 256, 128, 128) f32
    weight (320, 256, 3, 3)   f32
    bias   (320,)             f32
    param  (5,)               i32   per-group input-channel thresholds
the reference
    out = conv2d(x, weight * mask(param), stride 1, VALID) + bias
    out shape (8, 320, 126, 126) f32
where mask zeroes weight[o, i] for i < param[o // 64].

Strategy: data-parallel over batch — one image per NeuronCore (8 cores),
weights/bias replicated. Weight masking happens on the host (tiny, exact for
any runtime `param`).

PE layout ("spatial-as-M"): the stationary operand is an input-row patch
x[cin_block, row, dx:dx+126] (K=128 cin x M=126 pixels) and the moving
operand is the weight tap w[cin_block, dy, dx] (K=128 cin x N=320 couts).
Each output row accumulates 18 matmuls (2 cin blocks x 9 taps) into one
fp32 PSUM bank as [126 pix, 320 cout]; the DVE evacuates with a fused bias
add and the result is stored as [row, pix, cout] (contiguous), with the
final transpose to [cout, row, pix] done on the host.

Why: with couts as M (PSUM partitions), 320 couts force ceil(320/128)=3
passes of every x stream — the 64-wide last block runs at half PE width,
857k moving-rows/core. With pixels as M (126/128 used) and couts as the
free dim (N=320), it is 126x18 matmuls x 320 rows = 726k moving-rows/core,
and bf16 runs 1 row/cycle at 2.4 GHz regardless of free size (float32r
needs N>=256; bf16 also avoids its SBUF-bandwidth contention). bf16
inputs keep max-err ~2e-3 of ref absmax, well under the 2e-2 gate.
"""

import numpy as np

import concourse.mybir as mybir
import concourse.tile as tile
from concourse import bacc
from concourse.bass_utils import run_bass_kernel_spmd


def _ensure_axon_ntff_hook():
    """Best-effort: register the axon NTFF profile hook if the image's
    `antenv` stub lacks `axon_hooks` (concourse's trace path imports it
    unconditionally when BASS_TRACE is set). Purely optional — failures are
    ignored and tracing is simply unavailable."""
    try:
        import sys
        import types

        import antenv

        if "antenv.axon_hooks" in sys.modules:
            return
        mod = types.ModuleType("antenv.axon_hooks")
        _hook = [None]
        mod.set_axon_ntff_profile_hook = lambda h: _hook.__setitem__(0, h)
        mod.get_axon_ntff_profile_hook = lambda: _hook[0]
        sys.modules["antenv.axon_hooks"] = mod
        antenv.axon_hooks = mod
        from trn_agent_boot.trn_boot import _ntff_profile_via_ctypes

        mod.set_axon_ntff_profile_hook(
            _ntff_profile_via_ctypes("/opt/axon/libaxon_pjrt.so")
        )
    except Exception:
        pass


_ensure_axon_ntff_hook()

N_CORES = 8
P = 128
CIN, COUT, KH, KW = 256, 320, 3, 3
H = W = 128
HO = WO = 126
CB = CIN // P  # 2 cin blocks
NTAP = CB * KH * KW  # 18 accumulated matmuls per output row

_NC_CACHE = {}


def _build_nc():
    nc = bacc.Bacc("TRN2", target_bir_lowering=False, debug=False)
    f32 = mybir.dt.float32
    bf16 = mybir.dt.bfloat16

    x_d = nc.dram_tensor("x", [CIN, H, W], bf16, kind="ExternalInput").ap()
    w_d = nc.dram_tensor(
        "wt", [P, CB, KH, KW, COUT], bf16, kind="ExternalInput"
    ).ap()
    b_d = nc.dram_tensor("biasp", [P, COUT], f32, kind="ExternalInput").ap()
    o_d = nc.dram_tensor("out", [HO, WO, COUT], f32, kind="ExternalOutput").ap()

    # x viewed as [p, cb, h, w]: cin = cb*128 + p
    x_re = x_d.rearrange("(cb p) h w -> p cb h w", p=P)

    with tile.TileContext(nc) as tc:
        with (
            tc.tile_pool(name="wpool", bufs=1) as wpool,
            tc.tile_pool(name="opool", bufs=8) as opool,
            tc.tile_pool(name="psum", bufs=8, space="PSUM") as psum_pool,
        ):
            wt = wpool.tile([P, CB, KH, KW, COUT], bf16)
            bt = wpool.tile([P, COUT], f32)
            xt = wpool.tile([P, CB, H, W], bf16)

            # DMA staging across three HWDGE queues (sync + scalar + gpsimd),
            # ordered so the first matmuls can start as soon as their slices
            # land: each queue drains in program order, so first-needed goes
            # first. gpsimd carries the critical first weight taps (w(0,0)
            # split per dx so the very first matmul only waits on 82KB).
            nc.gpsimd.dma_start(wt[:, 0, 0, 0], w_d[:, 0, 0, 0])
            nc.gpsimd.dma_start(wt[:, 0, 0, 1], w_d[:, 0, 0, 1])
            nc.gpsimd.dma_start(wt[:, 0, 0, 2], w_d[:, 0, 0, 2])
            nc.gpsimd.dma_start(wt[:, 1, 0], w_d[:, 1, 0])
            nc.sync.dma_start(xt[:, 0, 0:3], x_re[:, 0, 0:3, :])
            nc.sync.dma_start(xt[:, 1, 0:3], x_re[:, 1, 0:3, :])
            nc.sync.dma_start(xt[:, 0, 3:8], x_re[:, 0, 3:8, :])
            nc.sync.dma_start(xt[:, 1, 3:8], x_re[:, 1, 3:8, :])
            nc.sync.dma_start(wt[:, 0, 1], w_d[:, 0, 1])
            nc.sync.dma_start(xt[:, 0, 8:16], x_re[:, 0, 8:16, :])
            nc.scalar.dma_start(wt[:, 1, 1], w_d[:, 1, 1])
            nc.scalar.dma_start(wt[:, 0, 2], w_d[:, 0, 2])
            nc.scalar.dma_start(wt[:, 1, 2], w_d[:, 1, 2])
            nc.scalar.dma_start(bt[:], b_d[:])
            nc.scalar.dma_start(xt[:, 1, 8:16], x_re[:, 1, 8:16, :])
            for r0 in range(16, H, 16):
                nc.sync.dma_start(
                    xt[:, 0, r0 : r0 + 16], x_re[:, 0, r0 : r0 + 16, :]
                )
                nc.scalar.dma_start(
                    xt[:, 1, r0 : r0 + 16], x_re[:, 1, r0 : r0 + 16, :]
                )

            def evac(r, ps):
                ot = opool.tile([P, COUT], f32, tag="ot")
                # evacuate PSUM -> SBUF with fused bias add (bias varies along
                # the free/cout dim, so tensor_tensor rather than ACT bias)
                nc.vector.tensor_tensor(
                    ot[:WO], ps[:WO], bt[:WO], mybir.AluOpType.add
                )
                (nc.sync if r % 2 == 0 else nc.scalar).dma_start(o_d[r], ot[:WO])

            # Warm-up sweep over output rows 0..5: the weight taps are still
            # streaming in, so go tap-chunk-major across 6 rows (each chunk
            # gets ~2.5us of matmuls, masking the next chunk's DMA), in the
            # order the chunks arrive on their queues. The inner row-block
            # split (0-2 then 3-5) lets the first matmuls run off the first
            # tiny x chunk (rows 0:3) before rows 3:8 have landed.
            WARM = 6
            phases = [(0, 0), (1, 0), (0, 1), (1, 1), (0, 2), (1, 2)]
            pss = [
                psum_pool.tile([P, COUT], f32, tag="ps", name=f"ps_w{t}")
                for t in range(WARM)
            ]
            for pi, (cb, dy) in enumerate(phases):
                for rblk in ((0, 1, 2), (3, 4, 5)):
                    for dx in range(KW):
                        for r in rblk:
                            nc.tensor.matmul(
                                pss[r][:WO],
                                xt[:, cb, r + dy, dx : dx + WO],
                                wt[:, cb, dy, dx, :],
                                start=(pi == 0 and dx == 0),
                                stop=(pi == len(phases) - 1 and dx == KW - 1),
                            )
            for r in range(WARM):
                evac(r, pss[r])

            # Steady state, input-row major: one stationary load per
            # (cin block, input row, dx) feeds up to 3 accumulating matmuls
            # (output rows i-dy), letting the PE reuse the loaded stationary
            # if codegen elides back-to-back identical LDWEIGHTS.
            open_ps = {}
            for i in range(WARM, H):
                for cb in range(CB):
                    for dx in range(KW):
                        lhs = xt[:, cb, i, dx : dx + WO]
                        for dy in range(KH):
                            r = i - dy
                            if r < WARM or r >= HO:
                                continue
                            if r not in open_ps:
                                open_ps[r] = psum_pool.tile(
                                    [P, COUT], f32, tag="ps", name=f"ps_{r}"
                                )
                            nc.tensor.matmul(
                                open_ps[r][:WO],
                                lhs,
                                wt[:, cb, dy, dx, :],
                                start=(cb == 0 and dx == 0 and dy == 0),
                                stop=(cb == CB - 1 and dx == KW - 1 and dy == KH - 1),
                            )
                rdone = i - 2
                if rdone >= WARM:
                    evac(rdone, open_ps.pop(rdone))
    nc.compile()
    return nc


def _get_nc():
    key = "bf16-spatial-m-v3"
    if key not in _NC_CACHE:
        _NC_CACHE[key] = _build_nc()
    return _NC_CACHE[key]


def _preprocess(x, weight, bias, param):
    import ml_dtypes

    bf16 = ml_dtypes.bfloat16
    x = np.asarray(x, dtype=np.float32)
    weight = np.asarray(weight, dtype=np.float32)
    bias = np.asarray(bias, dtype=np.float32)
    param = np.asarray(param)

    # host-side weight masking (group g of 64 output channels uses cin >=
    # param[g]); mask before the bf16 round so masked taps are exact zeros
    thresh = np.repeat(param.astype(np.int64), COUT // param.shape[0])  # [COUT]
    mask = (np.arange(CIN)[None, :] >= thresh[:, None]).astype(np.float32)
    wm = weight * mask[:, :, None, None]
    # moving-operand layout: [p, cb, kh, kw, cout]
    wT = np.ascontiguousarray(
        wm.reshape(COUT, CB, P, KH, KW).transpose(2, 1, 3, 4, 0)
    ).astype(bf16)
    xb = x.astype(bf16)
    biasp = np.ascontiguousarray(
        np.broadcast_to(bias[None, :], (P, COUT))
    )
    return xb, wT, biasp


def _postprocess(results):
    # per-core "out" is [HO, WO, COUT]; full output is [8, COUT, HO, WO]
    out = np.stack([r["out"] for r in results], axis=0)
    return np.ascontiguousarray(out.transpose(0, 3, 1, 2))


def kernel(x, weight, bias, param):
    xb, wT, biasp = _preprocess(x, weight, bias, param)
    nc = _get_nc()
    in_maps = [{"x": xb[i], "wt": wT, "biasp": biasp} for i in range(N_CORES)]
    res = run_bass_kernel_spmd(nc, in_maps, core_ids=list(range(N_CORES)))
    return _postprocess(res.results)


# revision 9
# speedup vs baseline: 1.2008x; 1.2008x over previous
"""Trainium2 Bass kernel for nn_ChannelGroupConvUneven.

Computes, for full inputs
    x      (8, 256, 128, 128) f32
    weight (320, 256, 3, 3)   f32
    bias   (320,)             f32
    param  (5,)               i32   per-group input-channel thresholds
the reference
    out = conv2d(x, weight * mask(param), stride 1, VALID) + bias
    out shape (8, 320, 126, 126) f32
where mask zeroes weight[o, i] for i < param[o // 64].

Strategy: data-parallel over batch — one image per NeuronCore (8 cores),
weights/bias replicated. Weight masking happens on the host (tiny, exact for
any runtime `param`).

PE layout ("spatial-as-M"): the stationary operand is an input-row patch
x[cin_block, row, dx:dx+126] (K=128 cin x M=126 pixels) and the moving
operand is the weight tap w[cin_block, dy, dx] (K=128 cin x N=320 couts).
Each output row accumulates 18 matmuls (2 cin blocks x 9 taps) into one
fp32 PSUM bank as [126 pix, 320 cout]; the DVE evacuates with a fused bias
add and the result is stored as [row, pix, cout] (contiguous), with the
final transpose to [cout, row, pix] done on the host.

Why: with couts as M (PSUM partitions), 320 couts force ceil(320/128)=3
passes of every x stream — the 64-wide last block runs at half PE width,
857k moving-rows/core. With pixels as M (126/128 used) and couts as the
free dim (N=320), it is 126x18 matmuls x 320 rows = 726k moving-rows/core,
and bf16 runs 1 row/cycle at 2.4 GHz regardless of free size (float32r
needs N>=256; bf16 also avoids its SBUF-bandwidth contention). bf16
inputs keep max-err ~2e-3 of ref absmax, well under the 2e-2 gate.
"""

import numpy as np

import concourse.mybir as mybir
import concourse.tile as tile
from concourse import bacc
from concourse.bass_utils import run_bass_kernel_spmd


def _ensure_axon_ntff_hook():
    """Best-effort: register the axon NTFF profile hook if the image's
    `antenv` stub lacks `axon_hooks` (concourse's trace path imports it
    unconditionally when BASS_TRACE is set). Purely optional — failures are
    ignored and tracing is simply unavailable."""
    try:
        import sys
        import types

        import antenv

        if "antenv.axon_hooks" in sys.modules:
            return
        mod = types.ModuleType("antenv.axon_hooks")
        _hook = [None]
        mod.set_axon_ntff_profile_hook = lambda h: _hook.__setitem__(0, h)
        mod.get_axon_ntff_profile_hook = lambda: _hook[0]
        sys.modules["antenv.axon_hooks"] = mod
        antenv.axon_hooks = mod
        from trn_agent_boot.trn_boot import _ntff_profile_via_ctypes

        mod.set_axon_ntff_profile_hook(
            _ntff_profile_via_ctypes("/opt/axon/libaxon_pjrt.so")
        )
    except Exception:
        pass


_ensure_axon_ntff_hook()

N_CORES = 8
P = 128
CIN, COUT, KH, KW = 256, 320, 3, 3
H = W = 128
HO = WO = 126
CB = CIN // P  # 2 cin blocks
NTAP = CB * KH * KW  # 18 accumulated matmuls per output row

_NC_CACHE = {}


def _build_nc():
    nc = bacc.Bacc("TRN2", target_bir_lowering=False, debug=False)
    f32 = mybir.dt.float32
    bf16 = mybir.dt.bfloat16

    x_d = nc.dram_tensor("x", [CIN, H, W], bf16, kind="ExternalInput").ap()
    w_d = nc.dram_tensor(
        "wt", [P, CB, KH, KW, COUT], bf16, kind="ExternalInput"
    ).ap()
    b_d = nc.dram_tensor("biasp", [P, COUT], f32, kind="ExternalInput").ap()
    o_d = nc.dram_tensor("out", [HO, WO, COUT], f32, kind="ExternalOutput").ap()

    # x viewed as [p, cb, h, w]: cin = cb*128 + p
    x_re = x_d.rearrange("(cb p) h w -> p cb h w", p=P)

    with tile.TileContext(nc) as tc:
        with (
            tc.tile_pool(name="wpool", bufs=1) as wpool,
            tc.tile_pool(name="opool", bufs=8) as opool,
            tc.tile_pool(name="psum", bufs=8, space="PSUM") as psum_pool,
        ):
            wt = wpool.tile([P, CB, KH, KW, COUT], bf16)
            bt = wpool.tile([P, COUT], f32)
            xt = wpool.tile([P, CB, H, W], bf16)

            # DMA staging across three HWDGE queues (sync + scalar + gpsimd),
            # ordered so the first matmuls can start as soon as their slices
            # land: each queue drains in program order, so first-needed goes
            # first. gpsimd carries the critical first weight taps (w(0,0)
            # split per dx so the very first matmul only waits on 82KB).
            nc.gpsimd.dma_start(wt[:, 0, 0, 0], w_d[:, 0, 0, 0])
            nc.gpsimd.dma_start(wt[:, 0, 0, 1], w_d[:, 0, 0, 1])
            nc.gpsimd.dma_start(wt[:, 0, 0, 2], w_d[:, 0, 0, 2])
            nc.gpsimd.dma_start(wt[:, 1, 0], w_d[:, 1, 0])
            nc.sync.dma_start(xt[:, 0, 0:3], x_re[:, 0, 0:3, :])
            nc.sync.dma_start(xt[:, 1, 0:3], x_re[:, 1, 0:3, :])
            nc.sync.dma_start(xt[:, 0, 3:8], x_re[:, 0, 3:8, :])
            nc.sync.dma_start(xt[:, 1, 3:8], x_re[:, 1, 3:8, :])
            nc.sync.dma_start(wt[:, 0, 1], w_d[:, 0, 1])
            nc.sync.dma_start(xt[:, 0, 8:16], x_re[:, 0, 8:16, :])
            nc.scalar.dma_start(wt[:, 1, 1], w_d[:, 1, 1])
            nc.scalar.dma_start(wt[:, 0, 2], w_d[:, 0, 2])
            nc.scalar.dma_start(wt[:, 1, 2], w_d[:, 1, 2])
            nc.scalar.dma_start(bt[:], b_d[:])
            nc.scalar.dma_start(xt[:, 1, 8:16], x_re[:, 1, 8:16, :])
            for r0 in range(16, H, 16):
                nc.sync.dma_start(
                    xt[:, 0, r0 : r0 + 16], x_re[:, 0, r0 : r0 + 16, :]
                )
                nc.scalar.dma_start(
                    xt[:, 1, r0 : r0 + 16], x_re[:, 1, r0 : r0 + 16, :]
                )

            def evac(r, ps):
                ot = opool.tile([P, COUT], f32, tag="ot")
                # evacuate PSUM -> SBUF with fused bias add (bias varies along
                # the free/cout dim, so tensor_tensor rather than ACT bias)
                nc.vector.tensor_tensor(
                    ot[:WO], ps[:WO], bt[:WO], mybir.AluOpType.add
                )
                if r == HO - 1:
                    # split the last row across both queues to shorten the tail
                    nc.sync.dma_start(o_d[r, 0:63], ot[0:63])
                    nc.scalar.dma_start(o_d[r, 63:WO], ot[63:WO])
                else:
                    (nc.sync if r % 2 == 0 else nc.scalar).dma_start(
                        o_d[r], ot[:WO]
                    )

            # Warm-up sweep over output rows 0..5: the weight taps are still
            # streaming in, so go tap-chunk-major across 6 rows (each chunk
            # gets ~2.5us of matmuls, masking the next chunk's DMA), in the
            # order the chunks arrive on their queues. The inner row-block
            # split (0-2 then 3-5) lets the first matmuls run off the first
            # tiny x chunk (rows 0:3) before rows 3:8 have landed.
            WARM = 6
            phases = [(0, 0), (1, 0), (0, 1), (1, 1), (0, 2), (1, 2)]
            pss = [
                psum_pool.tile([P, COUT], f32, tag="ps", name=f"ps_w{t}")
                for t in range(WARM)
            ]
            for pi, (cb, dy) in enumerate(phases):
                for rblk in ((0, 1, 2), (3, 4, 5)):
                    for dx in range(KW):
                        for r in rblk:
                            nc.tensor.matmul(
                                pss[r][:WO],
                                xt[:, cb, r + dy, dx : dx + WO],
                                wt[:, cb, dy, dx, :],
                                start=(pi == 0 and dx == 0),
                                stop=(pi == len(phases) - 1 and dx == KW - 1),
                            )
            for r in range(WARM):
                evac(r, pss[r])

            # Steady state: per output row, 18 accumulating matmuls into one
            # PSUM bank. Keeping one accumulation group per bank back-to-back
            # is what the PE pipelines best (a bank-interleaved i-major
            # variant measured 163ns/matmul vs 137ns for this order).
            for r in range(WARM, HO):
                ps = psum_pool.tile([P, COUT], f32, tag="ps")
                k = 0
                for cb in range(CB):
                    for dy in range(KH):
                        for dx in range(KW):
                            nc.tensor.matmul(
                                ps[:WO],
                                xt[:, cb, r + dy, dx : dx + WO],
                                wt[:, cb, dy, dx, :],
                                start=(k == 0),
                                stop=(k == NTAP - 1),
                            )
                            k += 1
                evac(r, ps)
    nc.compile()
    return nc


def _get_nc():
    key = "bf16-spatial-m-v4"
    if key not in _NC_CACHE:
        _NC_CACHE[key] = _build_nc()
    return _NC_CACHE[key]


def _preprocess(x, weight, bias, param):
    import ml_dtypes

    bf16 = ml_dtypes.bfloat16
    x = np.asarray(x, dtype=np.float32)
    weight = np.asarray(weight, dtype=np.float32)
    bias = np.asarray(bias, dtype=np.float32)
    param = np.asarray(param)

    # host-side weight masking (group g of 64 output channels uses cin >=
    # param[g]); mask before the bf16 round so masked taps are exact zeros
    thresh = np.repeat(param.astype(np.int64), COUT // param.shape[0])  # [COUT]
    mask = (np.arange(CIN)[None, :] >= thresh[:, None]).astype(np.float32)
    wm = weight * mask[:, :, None, None]
    # moving-operand layout: [p, cb, kh, kw, cout]
    wT = np.ascontiguousarray(
        wm.reshape(COUT, CB, P, KH, KW).transpose(2, 1, 3, 4, 0)
    ).astype(bf16)
    xb = x.astype(bf16)
    biasp = np.ascontiguousarray(
        np.broadcast_to(bias[None, :], (P, COUT))
    )
    return xb, wT, biasp


def _postprocess(results):
    # per-core "out" is [HO, WO, COUT]; full output is [8, COUT, HO, WO]
    out = np.stack([r["out"] for r in results], axis=0)
    return np.ascontiguousarray(out.transpose(0, 3, 1, 2))


def kernel(x, weight, bias, param):
    xb, wT, biasp = _preprocess(x, weight, bias, param)
    nc = _get_nc()
    in_maps = [{"x": xb[i], "wt": wT, "biasp": biasp} for i in range(N_CORES)]
    res = run_bass_kernel_spmd(nc, in_maps, core_ids=list(range(N_CORES)))
    return _postprocess(res.results)
